# revision 1
# baseline (speedup 1.0000x reference)
"""ChemiNet GNN message-passing forward on 8 Trainium2 NeuronCores (Bass/Tile).

Strategy (self-contained; shapes hardcoded from the problem spec):
  - Host: sort edges by destination node, cut the (sorted) molecule range into
    8 contiguous shards with ~equal edge counts. Each core owns its molecules'
    nodes and exactly the edges that point into them, so the scatter-add
    aggregation, BN-stat partials and pooling are all core-local. Only the
    BatchNorm statistics need a (tiny) AllReduce.
  - Device, per 128-edge tile: PE computes z = [edge_attr|1] @ [W1;b1] into
    PSUM; DVE fuses relu+mult ( y_i = max(z_i,0) * x_src[:,i] ) via
    scalar_tensor_tensor with a broadcast access pattern; PE contracts over the
    75 atom features with a PSUM-accumulated matmul whose stationary weight is
    the tile's "same-destination" selection matrix, which simultaneously
    performs the within-tile scatter-add. Tiles never share a destination node
    (host pads tiles to node boundaries), so per-tile results are final node
    sums, written to DRAM densely and gathered per node block by indirect DMA.
  - Node stage: h = relu(x@W_root + bias + agg) per 128-node block, BN partial
    sums via matmuls against a ones vector, AllReduce, then the affine+pool+
    readout fused into a few matmuls/vector ops.
"""

import numpy as np
import ml_dtypes

import concourse.bass as bass
import concourse.bacc as bacc
import concourse.mybir as mybir
import concourse.tile as tile

BF16 = ml_dtypes.bfloat16

N = 20000
E = 40000
FA = 75
FB = 12
C = 100
G = 1000
EPS = 1e-5
NCORES = 8
P = 128
GSLOT = 128          # molecule slots per core (pool matmul M dim)
CHI = 10             # atom features per PSUM chunk (2 banks)
NSUB = 30            # trailing features contracted on PE via sel*xg weights
SUB_I0 = FA - NSUB   # = 45
# chunk plan: (start feature, count); PE path for start >= SUB_I0
CHUNKS = [(45, 10), (55, 10), (65, 10),
          (0, 10), (10, 10), (20, 10), (30, 10), (40, 5)]
F = FA * C           # 7500

_prog_cache = {}
_exec_cache = {}


# --------------------------------------------------------------------------
# host-side preparation
# --------------------------------------------------------------------------

def _make_cuts(batch, dst):
    """8 contiguous molecule ranges with ~equal edge counts, <=GSLOT mols."""
    edges_per_mol = np.bincount(batch[dst], minlength=G)
    cum = np.concatenate([[0], np.cumsum(edges_per_mol)])
    cuts = [0]
    for c in range(1, NCORES):
        g = int(np.searchsorted(cum, c * E / NCORES))
        lo = max(cuts[-1] + 1, G - (NCORES - c) * GSLOT)
        hi = min(G - (NCORES - c), cuts[-1] + GSLOT)
        cuts.append(min(max(g, lo), hi))
    cuts.append(G)
    return cuts


def _prep(inputs):
    x = np.asarray(inputs["x"], np.float32)
    ea = np.asarray(inputs["edge_attr"], np.float32)
    W1 = np.asarray(inputs["W1"], np.float32)
    b1 = np.asarray(inputs["b1"], np.float32)
    W_root = np.asarray(inputs["W_root"], np.float32)
    bias = np.asarray(inputs["bias"], np.float32)
    gamma = np.asarray(inputs["gamma"], np.float32)
    beta = np.asarray(inputs["beta"], np.float32)
    W_out = np.asarray(inputs["W_out"], np.float32)
    b_out = np.asarray(inputs["b_out"], np.float32)
    ei = np.asarray(inputs["edge_index"]).astype(np.int64)
    batch = np.asarray(inputs["batch"]).astype(np.int64)
    src, dst = ei[0], ei[1]

    nms = np.searchsorted(batch, np.arange(G + 1))  # node start of each molecule
    cuts = _make_cuts(batch, dst)

    # first pass: per-core tile packing to find the global tile count
    packs = []
    for c in range(NCORES):
        glo, ghi = cuts[c], cuts[c + 1]
        nlo, nhi = int(nms[glo]), int(nms[ghi])
        nl = nhi - nlo
        emask = (dst >= nlo) & (dst < nhi)
        eids = np.nonzero(emask)[0]
        order = np.argsort(dst[eids], kind="stable")
        eids = eids[order]                       # edges sorted by dst
        deg = np.bincount(dst[eids] - nlo, minlength=nl)
        assert deg.max(initial=0) <= P
        # node-aligned greedy tiles
        tile_of_node = np.zeros(nl, np.int64)
        slot_of_node = np.zeros(nl, np.int64)    # first slot of the node's run
        t, fill = 0, 0
        for n in range(nl):
            d = int(deg[n])
            if fill + d > P:
                t += 1
                fill = 0
            tile_of_node[n] = t
            slot_of_node[n] = fill
            fill += d
        ntiles = t + 1
        packs.append((glo, ghi, nlo, nhi, nl, eids, deg, tile_of_node,
                      slot_of_node, ntiles))

    T = max(p[9] for p in packs)
    NB = max((p[4] + P - 1) // P for p in packs)
    dump_row = T * P

    in_maps = []
    mols = []
    for c in range(NCORES):
        (glo, ghi, nlo, nhi, nl, eids, deg, tile_of_node, slot_of_node,
         ntiles) = packs[c]
        nmol = ghi - glo
        assert nmol <= GSLOT and nl <= NB * P
        mols.append(nmol)

        # per-edge tile/slot
        csum = np.concatenate([[0], np.cumsum(deg)])
        # position of each (sorted) edge within its node's run
        pos_in_node = np.arange(len(eids)) - csum[dst[eids] - nlo]
        et = tile_of_node[dst[eids] - nlo]
        ek = slot_of_node[dst[eids] - nlo] + pos_in_node
        assert ek.max(initial=0) < P

        eaT = np.zeros((T, FB + 1, P), np.float32)
        eaT[et, :FB, ek] = ea[eids]
        eaT[et, FB, ek] = 1.0
        xg = np.zeros((T, P, FA), np.float32)
        xg[et, ek] = x[src[eids]]
        # same-destination selection matrix per tile
        dslot = np.full((T, P), -1, np.int64)
        dslot[et, ek] = dst[eids]
        sel = (dslot[:, :, None] == dslot[:, None, :]) & (dslot[:, :, None] >= 0)

        # gather index per node slot: first msg row of the node, or dump row
        gat = np.full((NB * P,), dump_row, np.int64)
        has_e = deg > 0
        gat[: nl][has_e] = (tile_of_node[has_e] * P + slot_of_node[has_e])
        xt = np.zeros((FA + 1, NB * P), np.float32)
        xt[:FA, :nl] = x[nlo:nhi].T
        xt[FA, :nl] = 1.0
        poolhot = np.zeros((NB, P, GSLOT), np.float32)
        s = np.arange(nl)
        poolhot[s // P, s % P, batch[nlo:nhi] - glo] = 1.0

        # PE-path weights for the last NSUB features: sel[k,m] * xg[k,i],
        # shipped pre-transposed as [T, P(k), NSUB*P(j,m)]
        xgf = xg  # f32 before cast
        sxg = (sel.astype(np.float32)[:, None, :, :] *
               xgf[:, :, SUB_I0:].transpose(0, 2, 1)[:, :, :, None])
        sxg = sxg.transpose(0, 2, 1, 3).reshape(T, P, NSUB * P)

        in_maps.append({
            "eat": eaT.astype(BF16),
            "selxg": sxg.astype(BF16),
            "xg": xg.astype(BF16),
            "sel": sel.astype(BF16),
            "gidx": gat.reshape(NB, P, 1).astype(np.int32),
            "xt": xt,
            "poolhot": poolhot,
            "w1b1": np.concatenate([W1, b1[None, :]], 0).astype(BF16),
            "wroot": np.concatenate([W_root, bias[None, :]], 0).astype(np.float32),
            "i100": np.eye(C, dtype=np.float32),
            "gb": np.stack([gamma, beta], 1).astype(np.float32),
            "woutb": np.broadcast_to(W_out[:, 0][None, :], (P, C)).copy(),
            "boutb": np.full((P, 1), float(b_out[0]), np.float32),
        })
    return in_maps, mols, T, NB


# --------------------------------------------------------------------------
# device program
# --------------------------------------------------------------------------

DEBUG_OUTS = False
WITH_COLLECTIVE = True
NUM_DEVICES = NCORES


def _build(T, NB):
    key = (T, NB, DEBUG_OUTS, WITH_COLLECTIVE, NUM_DEVICES)
    if key in _prog_cache:
        return _prog_cache[key]
    fp32 = mybir.dt.float32
    bf16 = mybir.dt.bfloat16
    nc = bacc.Bacc("TRN2", target_bir_lowering=False, debug=False,
                   num_devices=NUM_DEVICES)

    eat = nc.dram_tensor("eat", [T, FB + 1, P], bf16, kind="ExternalInput")
    xg = nc.dram_tensor("xg", [T, P, FA], bf16, kind="ExternalInput")
    sel = nc.dram_tensor("sel", [T, P, P], bf16, kind="ExternalInput")
    selxg = nc.dram_tensor("selxg", [T, P, NSUB * P], bf16,
                           kind="ExternalInput")
    gidx = nc.dram_tensor("gidx", [NB, P, 1], mybir.dt.int32, kind="ExternalInput")
    xt = nc.dram_tensor("xt", [FA + 1, NB * P], fp32, kind="ExternalInput")
    poolhot = nc.dram_tensor("poolhot", [NB, P, GSLOT], fp32, kind="ExternalInput")
    w1b1 = nc.dram_tensor("w1b1", [FB + 1, F], bf16, kind="ExternalInput")
    wroot = nc.dram_tensor("wroot", [FA + 1, C], fp32, kind="ExternalInput")
    i100 = nc.dram_tensor("i100", [C, C], fp32, kind="ExternalInput")
    gb = nc.dram_tensor("gb", [C, 2], fp32, kind="ExternalInput")
    woutb = nc.dram_tensor("woutb", [P, C], fp32, kind="ExternalInput")
    boutb = nc.dram_tensor("boutb", [P, 1], fp32, kind="ExternalInput")
    res = nc.dram_tensor("res", [P, 1], fp32, kind="ExternalOutput")
    if DEBUG_OUTS:
        msg_dbg = nc.dram_tensor("msg_dbg", [T * P + 1, C], fp32,
                                 kind="ExternalOutput")
        h_dbg = nc.dram_tensor("h_dbg", [NB * P, C], fp32,
                               kind="ExternalOutput")
        ss_dbg = nc.dram_tensor("ss_dbg", [C, 2], fp32, kind="ExternalOutput")
        gs_dbg = nc.dram_tensor("gs_dbg", [C, 2], fp32, kind="ExternalOutput")
        pl_dbg = nc.dram_tensor("pl_dbg", [P, C + 1], fp32,
                                kind="ExternalOutput")

    Alu = mybir.AluOpType
    Act = mybir.ActivationFunctionType

    with tile.TileContext(nc) as tc:
        with (
            tc.tile_pool(name="const", bufs=1) as cpool,
            tc.tile_pool(name="sbB", bufs=6) as sbB,
            tc.tile_pool(name="dram", bufs=1, space="DRAM") as dpool,
        ):
            msgd = dpool.tile([T * P + 1, C], fp32)
            stat_in = dpool.tile([C, 2], fp32)
            stat_out = dpool.tile([C, 2], fp32)

            tw1 = cpool.tile([FB + 1, F], bf16)
            nc.sync.dma_start(tw1[:], w1b1[:])
            zrow = cpool.tile([1, C], fp32)
            nc.vector.memset(zrow[:], 0.0)
            nc.sync.dma_start(msgd[T * P:T * P + 1, :], zrow[:])


            # ---------------- stage A: edge tiles ----------------
            with (
                tc.tile_pool(name="sbA", bufs=3) as sbA,
                tc.tile_pool(name="sbY", bufs=3) as sbY,
                tc.tile_pool(name="psZ", bufs=3, space="PSUM") as psZ,
                tc.tile_pool(name="psM", bufs=2, space="PSUM") as psM,
            ):
                for t in range(T):
                    tea = sbA.tile([FB + 1, P], bf16, tag="tea")
                    txg = sbA.tile([P, FA], bf16, tag="txg")
                    tsel = sbA.tile([P, P], bf16, tag="tsel")
                    tsxg = sbA.tile([P, NSUB * P], bf16, tag="tsxg")
                    nc.sync.dma_start(tea[:], eat[t])
                    nc.sync.dma_start(txg[:], xg[t])
                    nc.sync.dma_start(tsel[:], sel[t])
                    nc.sync.dma_start(tsxg[:], selxg[t])
                    msg_ps = psM.tile([P, C], fp32, space="PSUM", tag="msg")
                    ncdone = 0
                    for (i0, ni) in CHUNKS:
                        nj = (ni * C + 499) // 500          # 500-col groups
                        # one 512-f32 PSUM bank per 500-col matmul
                        zc = psZ.tile([P, 2, 512], fp32, space="PSUM", tag="z")
                        for j in range(nj):
                            j0 = j * 500
                            j1 = min(j0 + 500, ni * C)
                            nc.tensor.matmul(
                                zc[:, j, 0:j1 - j0], lhsT=tea[:],
                                rhs=tw1[:, i0 * C + j0: i0 * C + j1],
                                start=True, stop=True)
                        yc = sbY.tile([P, CHI * C], bf16, tag="y")
                        if i0 >= SUB_I0:
                            # PE path: relu-evict on ACT, contract+scatter on
                            # PE with per-feature sel*xg weights
                            nc.scalar.activation(
                                yc[:, :ni * C].rearrange(
                                    "p (j o) -> p j o", j=nj),
                                zc[:, 0:nj, 0:500], Act.Relu)
                            for i in range(ni):
                                gi = i0 + i
                                nc.tensor.matmul(
                                    msg_ps[:],
                                    lhsT=tsxg[:, (gi - SUB_I0) * P:
                                              (gi - SUB_I0 + 1) * P],
                                    rhs=yc[:, i * C:(i + 1) * C],
                                    start=(ncdone + i == 0),
                                    stop=(ncdone + i == FA - 1))
                        else:
                            nc.vector.scalar_tensor_tensor(
                                out=yc[:, :ni * C].rearrange(
                                    "p (j i o) -> p j i o", j=nj, i=5),
                                in0=zc[:, :, 0:500].rearrange(
                                    "p j (i o) -> p j i o", i=5)[:, 0:nj],
                                scalar=0.0,
                                in1=txg[:, i0:i0 + ni].rearrange(
                                    "p (j i) -> p j i", j=nj).to_broadcast(
                                    [P, nj, 5, C]),
                                op0=Alu.max, op1=Alu.mult)
                            for i in range(ni):
                                gi = i0 + i
                                nc.tensor.matmul(
                                    msg_ps[:], lhsT=tsel[:],
                                    rhs=yc[:, i * C:(i + 1) * C],
                                    start=(ncdone + i == 0),
                                    stop=(ncdone + i == FA - 1))
                        ncdone += ni
                    msg_sb = sbA.tile([P, C], fp32, tag="msgsb")
                    nc.scalar.copy(msg_sb[:], msg_ps[:])
                    nc.sync.dma_start(msgd[t * P:(t + 1) * P, :], msg_sb[:])

            # ---------------- stage B: node blocks ----------------
            with tc.tile_pool(name="psAcc", bufs=1, space="PSUM") as psAcc:
              with (
                tc.tile_pool(name="psB", bufs=2, space="PSUM") as psB,
              ):
                twroot = cpool.tile([FA + 1, C], fp32)
                nc.sync.dma_start(twroot[:], wroot[:])
                ones128 = cpool.tile([P, 1], fp32)
                nc.vector.memset(ones128[:], 1.0)
                s1_ps = psAcc.tile([C, 1], fp32, space="PSUM", tag="s1")
                s2_ps = psAcc.tile([C, 1], fp32, space="PSUM", tag="s2")
                pool_ps = psAcc.tile([GSLOT, C + 1], fp32, space="PSUM", tag="pool")

                for nb in range(NB):
                    txt = sbB.tile([FA + 1, P], fp32, tag="txt")
                    nc.sync.dma_start(txt[:], xt[:, nb * P:(nb + 1) * P])
                    tgi = sbB.tile([P, 1], mybir.dt.int32, tag="tgi")
                    nc.sync.dma_start(tgi[:], gidx[nb])
                    thp = sbB.tile([P, GSLOT], fp32, tag="thp")
                    nc.sync.dma_start(thp[:], poolhot[nb])

                    xwp = psB.tile([P, C], fp32, space="PSUM", tag="xw")
                    nc.tensor.matmul(xwp[:], lhsT=txt[:], rhs=twroot[:],
                                     start=True, stop=True)
                    tagg = sbB.tile([P, C], fp32, tag="tagg")
                    nc.gpsimd.indirect_dma_start(
                        out=tagg[:], out_offset=None, in_=msgd[:],
                        in_offset=bass.IndirectOffsetOnAxis(ap=tgi[:, :1], axis=0))
                    hpre = sbB.tile([P, C], fp32, tag="hpre")
                    nc.vector.tensor_tensor(out=hpre[:], in0=xwp[:], in1=tagg[:],
                                            op=Alu.add)
                    hx = sbB.tile([P, C + 1], fp32, tag="hx")
                    nc.scalar.activation(hx[:, :C], hpre[:], Act.Relu)
                    nc.vector.memset(hx[:, C:C + 1], 1.0)
                    h2 = sbB.tile([P, C], fp32, tag="h2")
                    nc.vector.tensor_tensor(out=h2[:], in0=hx[:, :C],
                                            in1=hx[:, :C], op=Alu.mult)
                    nc.tensor.matmul(s1_ps[:], lhsT=hx[:, :C], rhs=ones128[:],
                                     start=(nb == 0), stop=(nb == NB - 1))
                    nc.tensor.matmul(s2_ps[:], lhsT=h2[:], rhs=ones128[:],
                                     start=(nb == 0), stop=(nb == NB - 1))
                    nc.tensor.matmul(pool_ps[:], lhsT=thp[:], rhs=hx[:],
                                     start=(nb == 0), stop=(nb == NB - 1))
                    if DEBUG_OUTS:
                        nc.sync.dma_start(h_dbg[nb * P:(nb + 1) * P, :],
                                          hx[:, :C])

              # ---------------- stage C: stats + readout ----------------
              with tc.tile_pool(name="sbC", bufs=1) as sbC, \
                   tc.tile_pool(name="psC", bufs=1, space="PSUM") as psC:
                  ssum = sbC.tile([C, 2], fp32)
                  nc.vector.tensor_copy(ssum[:, 0:1], s1_ps[:])
                  nc.vector.tensor_copy(ssum[:, 1:2], s2_ps[:])
                  nc.sync.dma_start(stat_in[:], ssum[:])
                  if DEBUG_OUTS:
                      nc.sync.dma_start(ss_dbg[:], ssum[:])
                  gs = sbC.tile([C, 2], fp32)
                  if WITH_COLLECTIVE:
                      nc.gpsimd.collective_compute(
                          "AllReduce", Alu.add,
                          replica_groups=[list(range(NCORES))],
                          ins=[stat_in[:].opt()], outs=[stat_out[:].opt()])
                      nc.sync.dma_start(gs[:], stat_out[:])
                  else:
                      nc.sync.dma_start(gs[:], stat_in[:])
                  if DEBUG_OUTS:
                      nc.sync.dma_start(gs_dbg[:], gs[:])
                      for tt in range(T):
                          mtmp = sbC.tile([P, C], fp32, tag="mtmp")
                          nc.sync.dma_start(mtmp[:], msgd[tt * P:(tt + 1) * P, :])
                          nc.sync.dma_start(msg_dbg[tt * P:(tt + 1) * P, :], mtmp[:])
                      pltmp = sbC.tile([P, C + 1], fp32)
                      nc.vector.tensor_copy(pltmp[:], pool_ps[:])
                      nc.sync.dma_start(pl_dbg[:], pltmp[:])

                  tgb = sbC.tile([C, 2], fp32)
                  nc.sync.dma_start(tgb[:], gb[:])
                  ti100 = sbC.tile([C, C], fp32)
                  nc.sync.dma_start(ti100[:], i100[:])
                  twob = sbC.tile([P, C], fp32)
                  nc.sync.dma_start(twob[:], woutb[:])
                  tbo = sbC.tile([P, 1], fp32)
                  nc.sync.dma_start(tbo[:], boutb[:])

                  mean = sbC.tile([C, 1], fp32)
                  nc.vector.tensor_scalar_mul(mean[:], gs[:, 0:1], 1.0 / N)
                  nmean = sbC.tile([C, 1], fp32)
                  nc.vector.tensor_scalar_mul(nmean[:], mean[:], -1.0)
                  ex2 = sbC.tile([C, 1], fp32)
                  nc.vector.tensor_scalar_mul(ex2[:], gs[:, 1:2], 1.0 / N)
                  var = sbC.tile([C, 1], fp32)
                  nc.vector.scalar_tensor_tensor(
                      out=var[:], in0=nmean[:], scalar=mean[:], in1=ex2[:],
                      op0=Alu.mult, op1=Alu.add)
                  epst = sbC.tile([C, 1], fp32)
                  nc.vector.memset(epst[:], EPS)
                  sd = sbC.tile([C, 1], fp32)
                  nc.scalar.activation(sd[:], var[:], Act.Sqrt, bias=epst[:])
                  inv = sbC.tile([C, 1], fp32)
                  nc.vector.reciprocal(inv[:], sd[:])
                  ss = sbC.tile([C, 2], fp32)
                  nc.vector.tensor_tensor(out=ss[:, 0:1], in0=inv[:],
                                          in1=tgb[:, 0:1], op=Alu.mult)
                  nc.vector.scalar_tensor_tensor(
                      out=ss[:, 1:2], in0=nmean[:], scalar=ss[:, 0:1],
                      in1=tgb[:, 1:2], op0=Alu.mult, op1=Alu.add)
                  sT_ps = psC.tile([1, C], fp32, space="PSUM", tag="sT")
                  nc.tensor.matmul(sT_ps[:], lhsT=ss[:, 0:1], rhs=ti100[:],
                                   start=True, stop=True)
                  hT_ps = psC.tile([1, C], fp32, space="PSUM", tag="hT")
                  nc.tensor.matmul(hT_ps[:], lhsT=ss[:, 1:2], rhs=ti100[:],
                                   start=True, stop=True)
                  sT = sbC.tile([1, C], fp32)
                  nc.vector.tensor_copy(sT[:], sT_ps[:])
                  hT = sbC.tile([1, C], fp32)
                  nc.vector.tensor_copy(hT[:], hT_ps[:])
                  ones1 = sbC.tile([1, P], fp32)
                  nc.vector.memset(ones1[:], 1.0)
                  sB_ps = psC.tile([P, C], fp32, space="PSUM", tag="sB")
                  nc.tensor.matmul(sB_ps[:], lhsT=ones1[:], rhs=sT[:],
                                   start=True, stop=True)
                  hB_ps = psC.tile([P, C], fp32, space="PSUM", tag="hB")
                  nc.tensor.matmul(hB_ps[:], lhsT=ones1[:], rhs=hT[:],
                                   start=True, stop=True)
                  scaleB = sbC.tile([P, C], fp32)
                  nc.scalar.copy(scaleB[:], sB_ps[:])
                  shiftB = sbC.tile([P, C], fp32)
                  nc.scalar.copy(shiftB[:], hB_ps[:])
                  t1 = sbC.tile([P, C], fp32)
                  nc.vector.tensor_tensor(out=t1[:], in0=pool_ps[:, 0:C],
                                          in1=scaleB[:], op=Alu.mult)
                  pooled = sbC.tile([P, C], fp32)
                  nc.vector.scalar_tensor_tensor(
                      out=pooled[:], in0=shiftB[:],
                      scalar=pool_ps[:, C:C + 1], in1=t1[:],
                      op0=Alu.mult, op1=Alu.add)
                  dummy = sbC.tile([P, C], fp32)
                  acc = sbC.tile([P, 1], fp32)
                  nc.vector.scalar_tensor_tensor(
                      out=dummy[:], in0=pooled[:], scalar=0.0, in1=twob[:],
                      op0=Alu.max, op1=Alu.mult, accum_out=acc[:])
                  resv = sbC.tile([P, 1], fp32)
                  nc.vector.tensor_tensor(out=resv[:], in0=acc[:],
                                          in1=tbo[:], op=Alu.add)
                  nc.sync.dma_start(res[:], resv[:])

    nc.compile()
    _prog_cache[key] = nc
    return nc


# --------------------------------------------------------------------------
# execution via PJRT (axon), with a cached jitted executable for re-runs
# --------------------------------------------------------------------------

def _get_executable(nc):
    key = id(nc)
    if key in _exec_cache:
        return _exec_cache[key]
    import jax
    from jax.sharding import Mesh, PartitionSpec
    from jax.experimental.shard_map import shard_map
    from concourse import bass2jax

    bass2jax.install_neuronx_cc_hook()
    partition_name = (nc.partition_id_tensor.name
                      if nc.partition_id_tensor else None)
    in_names, out_names, out_avals, zero_outs = [], [], [], []
    for alloc in nc.m.functions[0].allocations:
        if not isinstance(alloc, mybir.MemoryLocationSet):
            continue
        name = alloc.memorylocations[0].name
        if alloc.kind == "ExternalInput":
            if name != partition_name:
                in_names.append(name)
        elif alloc.kind == "ExternalOutput":
            shape = tuple(alloc.tensor_shape)
            dtype = mybir.dt.np(alloc.dtype)
            out_names.append(name)
            out_avals.append(jax.core.ShapedArray(shape, dtype))
            zero_outs.append(np.zeros(shape, dtype))
    n_params = len(in_names)
    all_in_names = list(in_names) + list(out_names)
    if partition_name is not None:
        all_in_names.append(partition_name)
    donate = tuple(range(n_params, n_params + len(out_names)))

    def _body(*args):
        operands = list(args)
        if partition_name is not None:
            operands.append(bass2jax.partition_id_tensor())
        outs = bass2jax._bass_exec_p.bind(
            *operands,
            out_avals=tuple(out_avals),
            in_names=tuple(all_in_names),
            out_names=tuple(out_names),
            lowering_input_output_aliases=(),
            sim_require_finite=True,
            sim_require_nnan=True,
            nc=nc)
        return tuple(outs)

    devices = jax.devices()[:NCORES]
    mesh = Mesh(np.asarray(devices), ("core",))
    in_specs = (PartitionSpec("core"),) * (n_params + len(out_names))
    out_specs = (PartitionSpec("core"),) * len(out_names)
    sharded = jax.jit(
        shard_map(_body, mesh=mesh, in_specs=in_specs, out_specs=out_specs,
                  check_rep=False),
        donate_argnums=donate, keep_unused=True)
    bundle = (sharded, in_names, out_names, out_avals, zero_outs, mesh)
    _exec_cache[key] = bundle
    return bundle


def _concat_inputs(bundle, in_maps, device_put=False):
    import jax
    from jax.sharding import NamedSharding, PartitionSpec
    sharded, in_names, out_names, out_avals, zero_outs, mesh = bundle
    concat_in = [np.concatenate([np.asarray(m[name]) for m in in_maps], axis=0)
                 for name in in_names]
    if device_put:
        sh = NamedSharding(mesh, PartitionSpec("core"))
        concat_in = [jax.device_put(a, sh) for a in concat_in]
    return concat_in


def _run_exec(bundle, concat_in):
    sharded, in_names, out_names, out_avals, zero_outs, mesh = bundle
    concat_zeros = [np.zeros((NCORES * z.shape[0], *z.shape[1:]), z.dtype)
                    for z in zero_outs]
    out_arrs = sharded(*concat_in, *concat_zeros)
    out_arrs = [np.asarray(a) for a in out_arrs]
    return [
        {name: out_arrs[i].reshape(NCORES, *out_avals[i].shape)[c]
         for i, name in enumerate(out_names)}
        for c in range(NCORES)
    ]


def _prep_and_compile(inputs):
    in_maps, mols, T, NB = _prep(inputs)
    nc = _build(T, NB)
    bundle = _get_executable(nc)
    return in_maps, mols, bundle


def kernel(**inputs) -> np.ndarray:
    in_maps, mols, bundle = _prep_and_compile(inputs)
    results = _run_exec(bundle, _concat_inputs(bundle, in_maps))
    out = np.concatenate(
        [results[c]["res"][: mols[c], :] for c in range(NCORES)], axis=0)
    return out.astype(np.float32)


if __name__ == "__main__":
    import jax
    cpu = jax.local_devices(backend="cpu")[0]
    with jax.default_device(cpu):
        import reference
        inputs = {k: np.asarray(v) for k, v in reference.setup_inputs().items()}
        expected = np.asarray(reference.reference(**inputs))
    actual = kernel(**inputs)
    err = np.abs(actual - expected).max() / np.abs(expected).max()
    print("shapes", actual.shape, expected.shape)
    print("Relative error:", err)



# revision 2
# speedup vs baseline: 97.2884x; 97.2884x over previous
"""ChemiNet GNN message-passing forward on 8 Trainium2 NeuronCores (Bass/Tile).

Strategy (self-contained; shapes hardcoded from the problem spec):
  - Host: cut the (sorted) molecule range into 8 contiguous shards with
    ~equal edge counts. Each core owns its molecules' nodes and exactly the
    edges that point into them (edges sorted by dst), so the scatter-add
    aggregation, BN-stat partials and pooling are all core-local. Only the
    BatchNorm statistics need a (tiny) AllReduce.
  - Device, per 128-edge tile (full packing, no node alignment):
      PE:  z = [edge_attr|1]^T-stationary @ [W1;b1]  (K=14, bf16) into PSUM,
           5 chunks x 15 features (3 banks each, ping-pong).
      DVE: y = max(z,0) * x_src  via one fused scalar_tensor_tensor per
           chunk (x broadcast over the 100 output channels), bf16 to SBUF.
      PE:  per-feature matmul msg_ps[slot,o] += sel^T @ y_f with a SINGLE
           shared one-hot slot matrix as stationary (no per-feature weight
           reloads); PSUM accumulation performs both the feature reduction
           and the within-tile scatter-add.
    Since tiles are packed to exactly 128 edges, a node's (dst-sorted) edge
    run can straddle two tiles; the node stage gathers up to 2 msg rows per
    node and adds them.
  - Node stage: h = relu(x@W_root + bias + agg) per 128-node block, BN
    partial sums via matmuls against ones, AllReduce, then affine + pool +
    readout fused into a few matmuls/vector ops.
"""

import numpy as np
import ml_dtypes

import concourse.bass as bass
import concourse.bacc as bacc
import concourse.mybir as mybir
import concourse.tile as tile

BF16 = ml_dtypes.bfloat16

N = 20000
E = 40000
FA = 75
FB = 12
C = 100
G = 1000
EPS = 1e-5
NCORES = 8
P = 128
GSLOT = 128          # molecule slots per core (pool matmul M dim)
F = FA * C           # 7500
NCH = 5              # z chunks per tile
FCH = 15             # features per chunk (NCH*FCH == FA)

_prog_cache = {}


# --------------------------------------------------------------------------
# host-side preparation
# --------------------------------------------------------------------------

def _make_cuts(batch, dst):
    """8 contiguous molecule ranges with ~equal edge counts, <=GSLOT mols."""
    edges_per_mol = np.bincount(batch[dst], minlength=G)
    cum = np.concatenate([[0], np.cumsum(edges_per_mol)])
    cuts = [0]
    for c in range(1, NCORES):
        g = int(np.searchsorted(cum, c * E / NCORES))
        lo = max(cuts[-1] + 1, G - (NCORES - c) * GSLOT)
        hi = min(G - (NCORES - c), cuts[-1] + GSLOT)
        cuts.append(min(max(g, lo), hi))
    cuts.append(G)
    return cuts


def _prep(inputs):
    x = np.asarray(inputs["x"], np.float32)
    ea = np.asarray(inputs["edge_attr"], np.float32)
    W1 = np.asarray(inputs["W1"], np.float32)
    b1 = np.asarray(inputs["b1"], np.float32)
    W_root = np.asarray(inputs["W_root"], np.float32)
    bias = np.asarray(inputs["bias"], np.float32)
    gamma = np.asarray(inputs["gamma"], np.float32)
    beta = np.asarray(inputs["beta"], np.float32)
    W_out = np.asarray(inputs["W_out"], np.float32)
    b_out = np.asarray(inputs["b_out"], np.float32)
    ei = np.asarray(inputs["edge_index"]).astype(np.int64)
    batch = np.asarray(inputs["batch"]).astype(np.int64)
    src, dst = ei[0], ei[1]

    nms = np.searchsorted(batch, np.arange(G + 1))  # node start of each mol
    cuts = _make_cuts(batch, dst)

    packs = []
    for c in range(NCORES):
        glo, ghi = cuts[c], cuts[c + 1]
        nlo, nhi = int(nms[glo]), int(nms[ghi])
        emask = (dst >= nlo) & (dst < nhi)
        eids = np.nonzero(emask)[0]
        order = np.argsort(dst[eids], kind="stable")
        eids = eids[order]                       # edges sorted by dst
        packs.append((glo, ghi, nlo, nhi, eids))

    T = max((len(p[4]) + P - 1) // P for p in packs)
    NB = max((p[3] - p[2] + P - 1) // P for p in packs)
    dump_row = T * P

    in_maps = []
    mols = []
    for c in range(NCORES):
        glo, ghi, nlo, nhi, eids = packs[c]
        nmol = ghi - glo
        nl = nhi - nlo
        ne = len(eids)
        assert nmol <= GSLOT and nl <= NB * P
        mols.append(nmol)

        ed = dst[eids] - nlo                     # local dst, sorted
        es = src[eids]
        et = np.arange(ne) // P                  # tile of each edge
        # slot of each edge's dst within its tile: rank of first occurrence
        # (dsts sorted => slot = ed - first ed in tile? no: slots dense)
        slot = np.zeros(ne, np.int64)
        for t in range(0, ne, P):
            seg = ed[t:t + P]
            uniq, inv = np.unique(seg, return_inverse=True)
            slot[t:t + P] = inv                  # dense slot ids, sorted order
        ek = slot

        eaT = np.zeros((T, FB + 2, P), np.float32)
        eaT[et, :FB, np.arange(ne) % P] = ea[eids]
        eaT[et, FB, np.arange(ne) % P] = 1.0
        xg = np.zeros((T, P, FA + 1), np.float32)
        xg[et, np.arange(ne) % P, :FA] = x[es]
        # one-hot slot matrix per tile: sel[k, m] = 1 if edge k in slot m
        sel = np.zeros((T, P, P), np.float32)
        sel[et, np.arange(ne) % P, ek] = 1.0

        # per-node gather rows (up to 2 tiles can hold a node's edges)
        gat1 = np.full((NB * P,), dump_row, np.int64)
        gat2 = np.full((NB * P,), dump_row, np.int64)
        if ne:
            first = np.searchsorted(ed, np.arange(nl), side="left")
            last = np.searchsorted(ed, np.arange(nl), side="right") - 1
            has = last >= first
            nz = np.nonzero(has)[0]
            t1 = et[first[nz]]
            t2 = et[last[nz]]
            s1 = ek[first[nz]]
            s2 = ek[last[nz]]
            gat1[nz] = t1 * P + s1
            two = t2 != t1
            gat2[nz[two]] = t2[two] * P + s2[two]

        xt = np.zeros((FA + 1, NB * P), np.float32)
        xt[:FA, :nl] = x[nlo:nhi].T
        xt[FA, :nl] = 1.0
        poolhot = np.zeros((NB, P, GSLOT), np.float32)
        s = np.arange(nl)
        poolhot[s // P, s % P, batch[nlo:nhi] - glo] = 1.0

        in_maps.append({
            "eat": eaT.astype(BF16),
            "xg": xg.astype(BF16),
            "sel": sel.astype(BF16),
            "gidx": gat1.reshape(NB, P, 1).astype(np.int32),
            "gidx2": gat2.reshape(NB, P, 1).astype(np.int32),
            "xt": xt,
            "poolhot": poolhot,
            "w1b1": np.concatenate(
                [W1, b1[None, :], np.zeros((1, F), np.float32)], 0
            ).astype(BF16),
            "wroot": np.concatenate([W_root, bias[None, :]], 0).astype(
                np.float32),
            "i100": np.eye(C, dtype=np.float32),
            "gb": np.stack([gamma, beta], 1).astype(np.float32),
            "woutb": np.broadcast_to(W_out[:, 0][None, :], (P, C)).copy(),
            "boutb": np.full((P, 1), float(b_out[0]), np.float32),
        })
    return in_maps, mols, T, NB


# --------------------------------------------------------------------------
# device program
# --------------------------------------------------------------------------

WITH_COLLECTIVE = True
NUM_DEVICES = NCORES


def _build(T, NB):
    key = (T, NB, WITH_COLLECTIVE, NUM_DEVICES)
    if key in _prog_cache:
        return _prog_cache[key]
    fp32 = mybir.dt.float32
    bf16 = mybir.dt.bfloat16
    nc = bacc.Bacc("TRN2", target_bir_lowering=False, debug=False,
                   num_devices=NUM_DEVICES)

    KD = FB + 2   # 14: FB features + bias row + zero pad row
    eat = nc.dram_tensor("eat", [T, KD, P], bf16, kind="ExternalInput")
    xg = nc.dram_tensor("xg", [T, P, FA + 1], bf16, kind="ExternalInput")
    sel = nc.dram_tensor("sel", [T, P, P], bf16, kind="ExternalInput")
    gidx = nc.dram_tensor("gidx", [NB, P, 1], mybir.dt.int32,
                          kind="ExternalInput")
    gidx2 = nc.dram_tensor("gidx2", [NB, P, 1], mybir.dt.int32,
                           kind="ExternalInput")
    xt = nc.dram_tensor("xt", [FA + 1, NB * P], fp32, kind="ExternalInput")
    poolhot = nc.dram_tensor("poolhot", [NB, P, GSLOT], fp32,
                             kind="ExternalInput")
    w1b1 = nc.dram_tensor("w1b1", [KD, F], bf16, kind="ExternalInput")
    wroot = nc.dram_tensor("wroot", [FA + 1, C], fp32, kind="ExternalInput")
    i100 = nc.dram_tensor("i100", [C, C], fp32, kind="ExternalInput")
    gb = nc.dram_tensor("gb", [C, 2], fp32, kind="ExternalInput")
    woutb = nc.dram_tensor("woutb", [P, C], fp32, kind="ExternalInput")
    boutb = nc.dram_tensor("boutb", [P, 1], fp32, kind="ExternalInput")
    res = nc.dram_tensor("res", [P, 1], fp32, kind="ExternalOutput")

    Alu = mybir.AluOpType
    Act = mybir.ActivationFunctionType

    with tile.TileContext(nc) as tc:
        with (
            tc.tile_pool(name="const", bufs=1) as cpool,
            tc.tile_pool(name="sbB", bufs=6) as sbB,
            tc.tile_pool(name="dram", bufs=1, space="DRAM") as dpool,
        ):
            msgd = dpool.tile([T * P + 1, C], fp32)
            stat_in = dpool.tile([C, 2], fp32)
            stat_out = dpool.tile([C, 2], fp32)

            tw1 = cpool.tile([KD, F], bf16)
            nc.sync.dma_start(tw1[:], w1b1[:])
            zrow = cpool.tile([1, C], fp32)
            nc.vector.memset(zrow[:], 0.0)
            nc.sync.dma_start(msgd[T * P:T * P + 1, :], zrow[:])

            # ---------------- stage A: edge tiles ----------------
            with (
                tc.tile_pool(name="sbA", bufs=3) as sbA,
                tc.tile_pool(name="sbY", bufs=2) as sbY,
                tc.tile_pool(name="psZ", bufs=2, space="PSUM") as psZ,
                tc.tile_pool(name="psM", bufs=2, space="PSUM") as psM,
            ):
                for t in range(T):
                    tea = sbA.tile([KD, P], bf16, tag="tea")
                    txg = sbA.tile([P, FA + 1], bf16, tag="txg")
                    tsel = sbA.tile([P, P], bf16, tag="tsel")
                    nc.sync.dma_start(tea[:], eat[t])
                    nc.sync.dma_start(txg[:], xg[t])
                    nc.sync.dma_start(tsel[:], sel[t])
                    msg_ps = psM.tile([P, 512], fp32, space="PSUM", tag="msg")
                    for ci in range(NCH):
                        zc = psZ.tile([P, 3, 512], fp32, space="PSUM", tag="z")
                        for j in range(3):
                            j0 = ci * FCH * C + j * 500
                            nc.tensor.matmul(
                                zc[:, j, 0:500], lhsT=tea[:],
                                rhs=tw1[:, j0:j0 + 500],
                                start=True, stop=True)
                        yc = sbY.tile([P, FCH * C], bf16, tag="y")
                        # y = max(z,0) * x_src[:, i]  (broadcast over o)
                        nc.vector.scalar_tensor_tensor(
                            out=yc[:].rearrange("p (j i o) -> p j i o",
                                                j=3, i=5),
                            in0=zc[:, :, 0:500].rearrange(
                                "p j (i o) -> p j i o", i=5),
                            scalar=0.0,
                            in1=txg[:, ci * FCH:(ci + 1) * FCH].rearrange(
                                "p (j i) -> p j i", j=3).to_broadcast(
                                [P, 3, 5, C]),
                            op0=Alu.max, op1=Alu.mult)
                        for f in range(FCH):
                            nc.tensor.matmul(
                                msg_ps[:, 0:C], lhsT=tsel[:],
                                rhs=yc[:, f * C:(f + 1) * C],
                                start=(ci == 0 and f == 0),
                                stop=(ci == NCH - 1 and f == FCH - 1))
                    msg_sb = sbA.tile([P, C], fp32, tag="msgsb")
                    nc.scalar.copy(msg_sb[:], msg_ps[:, 0:C])
                    nc.sync.dma_start(msgd[t * P:(t + 1) * P, :], msg_sb[:])

            # ---------------- stage B: node blocks ----------------
            with tc.tile_pool(name="psAcc", bufs=1, space="PSUM") as psAcc:
              with (
                tc.tile_pool(name="psB", bufs=2, space="PSUM") as psB,
              ):
                twroot = cpool.tile([FA + 1, C], fp32)
                nc.sync.dma_start(twroot[:], wroot[:])
                ones128 = cpool.tile([P, 1], fp32)
                nc.vector.memset(ones128[:], 1.0)
                s1_ps = psAcc.tile([C, 1], fp32, space="PSUM", tag="s1")
                s2_ps = psAcc.tile([C, 1], fp32, space="PSUM", tag="s2")
                pool_ps = psAcc.tile([GSLOT, C + 1], fp32, space="PSUM",
                                     tag="pool")

                for nb in range(NB):
                    txt = sbB.tile([FA + 1, P], fp32, tag="txt")
                    nc.sync.dma_start(txt[:], xt[:, nb * P:(nb + 1) * P])
                    tgi = sbB.tile([P, 1], mybir.dt.int32, tag="tgi")
                    nc.sync.dma_start(tgi[:], gidx[nb])
                    tgi2 = sbB.tile([P, 1], mybir.dt.int32, tag="tgi2")
                    nc.sync.dma_start(tgi2[:], gidx2[nb])
                    thp = sbB.tile([P, GSLOT], fp32, tag="thp")
                    nc.sync.dma_start(thp[:], poolhot[nb])

                    xwp = psB.tile([P, C], fp32, space="PSUM", tag="xw")
                    nc.tensor.matmul(xwp[:], lhsT=txt[:], rhs=twroot[:],
                                     start=True, stop=True)
                    tagg = sbB.tile([P, C], fp32, tag="tagg")
                    nc.gpsimd.indirect_dma_start(
                        out=tagg[:], out_offset=None, in_=msgd[:],
                        in_offset=bass.IndirectOffsetOnAxis(
                            ap=tgi[:, :1], axis=0))
                    tagg2 = sbB.tile([P, C], fp32, tag="tagg2")
                    nc.gpsimd.indirect_dma_start(
                        out=tagg2[:], out_offset=None, in_=msgd[:],
                        in_offset=bass.IndirectOffsetOnAxis(
                            ap=tgi2[:, :1], axis=0))
                    hpre = sbB.tile([P, C], fp32, tag="hpre")
                    nc.vector.tensor_tensor(out=hpre[:], in0=xwp[:],
                                            in1=tagg[:], op=Alu.add)
                    hpre2 = sbB.tile([P, C], fp32, tag="hpre2")
                    nc.vector.tensor_tensor(out=hpre2[:], in0=hpre[:],
                                            in1=tagg2[:], op=Alu.add)
                    hx = sbB.tile([P, C + 1], fp32, tag="hx")
                    nc.scalar.activation(hx[:, :C], hpre2[:], Act.Relu)
                    nc.vector.memset(hx[:, C:C + 1], 1.0)
                    h2 = sbB.tile([P, C], fp32, tag="h2")
                    nc.vector.tensor_tensor(out=h2[:], in0=hx[:, :C],
                                            in1=hx[:, :C], op=Alu.mult)
                    nc.tensor.matmul(s1_ps[:], lhsT=hx[:, :C], rhs=ones128[:],
                                     start=(nb == 0), stop=(nb == NB - 1))
                    nc.tensor.matmul(s2_ps[:], lhsT=h2[:], rhs=ones128[:],
                                     start=(nb == 0), stop=(nb == NB - 1))
                    nc.tensor.matmul(pool_ps[:], lhsT=thp[:], rhs=hx[:],
                                     start=(nb == 0), stop=(nb == NB - 1))

              # ---------------- stage C: stats + readout ----------------
              with tc.tile_pool(name="sbC", bufs=1) as sbC, \
                   tc.tile_pool(name="psC", bufs=1, space="PSUM") as psC:
                  ssum = sbC.tile([C, 2], fp32)
                  nc.vector.tensor_copy(ssum[:, 0:1], s1_ps[:])
                  nc.vector.tensor_copy(ssum[:, 1:2], s2_ps[:])
                  nc.sync.dma_start(stat_in[:], ssum[:])
                  gs = sbC.tile([C, 2], fp32)
                  if WITH_COLLECTIVE:
                      nc.gpsimd.collective_compute(
                          "AllReduce", Alu.add,
                          replica_groups=[list(range(NCORES))],
                          ins=[stat_in[:].opt()], outs=[stat_out[:].opt()])
                      nc.sync.dma_start(gs[:], stat_out[:])
                  else:
                      nc.sync.dma_start(gs[:], stat_in[:])

                  tgb = sbC.tile([C, 2], fp32)
                  nc.sync.dma_start(tgb[:], gb[:])
                  ti100 = sbC.tile([C, C], fp32)
                  nc.sync.dma_start(ti100[:], i100[:])
                  twob = sbC.tile([P, C], fp32)
                  nc.sync.dma_start(twob[:], woutb[:])
                  tbo = sbC.tile([P, 1], fp32)
                  nc.sync.dma_start(tbo[:], boutb[:])

                  mean = sbC.tile([C, 1], fp32)
                  nc.vector.tensor_scalar_mul(mean[:], gs[:, 0:1], 1.0 / N)
                  nmean = sbC.tile([C, 1], fp32)
                  nc.vector.tensor_scalar_mul(nmean[:], mean[:], -1.0)
                  ex2 = sbC.tile([C, 1], fp32)
                  nc.vector.tensor_scalar_mul(ex2[:], gs[:, 1:2], 1.0 / N)
                  var = sbC.tile([C, 1], fp32)
                  nc.vector.scalar_tensor_tensor(
                      out=var[:], in0=nmean[:], scalar=mean[:], in1=ex2[:],
                      op0=Alu.mult, op1=Alu.add)
                  epst = sbC.tile([C, 1], fp32)
                  nc.vector.memset(epst[:], EPS)
                  sd = sbC.tile([C, 1], fp32)
                  nc.scalar.activation(sd[:], var[:], Act.Sqrt, bias=epst[:])
                  inv = sbC.tile([C, 1], fp32)
                  nc.vector.reciprocal(inv[:], sd[:])
                  ss = sbC.tile([C, 2], fp32)
                  nc.vector.tensor_tensor(out=ss[:, 0:1], in0=inv[:],
                                          in1=tgb[:, 0:1], op=Alu.mult)
                  nc.vector.scalar_tensor_tensor(
                      out=ss[:, 1:2], in0=nmean[:], scalar=ss[:, 0:1],
                      in1=tgb[:, 1:2], op0=Alu.mult, op1=Alu.add)
                  sT_ps = psC.tile([1, C], fp32, space="PSUM", tag="sT")
                  nc.tensor.matmul(sT_ps[:], lhsT=ss[:, 0:1], rhs=ti100[:],
                                   start=True, stop=True)
                  hT_ps = psC.tile([1, C], fp32, space="PSUM", tag="hT")
                  nc.tensor.matmul(hT_ps[:], lhsT=ss[:, 1:2], rhs=ti100[:],
                                   start=True, stop=True)
                  sT = sbC.tile([1, C], fp32)
                  nc.vector.tensor_copy(sT[:], sT_ps[:])
                  hT = sbC.tile([1, C], fp32)
                  nc.vector.tensor_copy(hT[:], hT_ps[:])
                  ones1 = sbC.tile([1, P], fp32)
                  nc.vector.memset(ones1[:], 1.0)
                  sB_ps = psC.tile([P, C], fp32, space="PSUM", tag="sB")
                  nc.tensor.matmul(sB_ps[:], lhsT=ones1[:], rhs=sT[:],
                                   start=True, stop=True)
                  hB_ps = psC.tile([P, C], fp32, space="PSUM", tag="hB")
                  nc.tensor.matmul(hB_ps[:], lhsT=ones1[:], rhs=hT[:],
                                   start=True, stop=True)
                  scaleB = sbC.tile([P, C], fp32)
                  nc.scalar.copy(scaleB[:], sB_ps[:])
                  shiftB = sbC.tile([P, C], fp32)
                  nc.scalar.copy(shiftB[:], hB_ps[:])
                  t1 = sbC.tile([P, C], fp32)
                  nc.vector.tensor_tensor(out=t1[:], in0=pool_ps[:, 0:C],
                                          in1=scaleB[:], op=Alu.mult)
                  pooled = sbC.tile([P, C], fp32)
                  nc.vector.scalar_tensor_tensor(
                      out=pooled[:], in0=shiftB[:],
                      scalar=pool_ps[:, C:C + 1], in1=t1[:],
                      op0=Alu.mult, op1=Alu.add)
                  dummy = sbC.tile([P, C], fp32)
                  acc = sbC.tile([P, 1], fp32)
                  nc.vector.scalar_tensor_tensor(
                      out=dummy[:], in0=pooled[:], scalar=0.0, in1=twob[:],
                      op0=Alu.max, op1=Alu.mult, accum_out=acc[:])
                  resv = sbC.tile([P, 1], fp32)
                  nc.vector.tensor_tensor(out=resv[:], in0=acc[:],
                                          in1=tbo[:], op=Alu.add)
                  nc.sync.dma_start(res[:], resv[:])

    nc.compile()
    _prog_cache[key] = nc
    return nc


# --------------------------------------------------------------------------
# execution (run_bass_kernel_spmd; under axon this routes through PJRT)
# --------------------------------------------------------------------------

def _prep_and_build(inputs):
    in_maps, mols, T, NB = _prep(inputs)
    nc = _build(T, NB)
    return in_maps, mols, nc


def _unshard(results, mols):
    out = np.concatenate(
        [results[c]["res"][: mols[c], :] for c in range(NCORES)], axis=0)
    return out.astype(np.float32)


def kernel(**inputs) -> np.ndarray:
    from concourse import bass_utils
    in_maps, mols, nc = _prep_and_build(inputs)
    br = bass_utils.run_bass_kernel_spmd(
        nc, in_maps, core_ids=list(range(NCORES)))
    return _unshard(br.results, mols)


if __name__ == "__main__":
    import jax
    cpu = jax.local_devices(backend="cpu")[0]
    with jax.default_device(cpu):
        import reference
        inputs = {k: np.asarray(v) for k, v in reference.setup_inputs().items()}
        expected = np.asarray(reference.reference(**inputs))
    actual = kernel(**inputs)
    err = np.abs(actual - expected).max() / np.abs(expected).max()
    print("shapes", actual.shape, expected.shape)
    print("Relative error:", err)
